# revision 34
# baseline (speedup 1.0000x reference)
"""BertSelfAttention (B=4, S=4096, D=512) on 8 TRN2 NeuronCores.

Sharding: core c handles batch b = c//2 and query-row half h = c%2
(2048 q rows). Each core projects K/V for its own 2048 keys and the
halves are exchanged within each core pair via AllGather.

Layout trick: everything is computed transposed so no on-device
transposes are needed:
  QT[e, q] = Wq @ x.T          (lhsT = WqT chunks, rhs = xT chunks)
  KT[e, k] = Wk @ x.T
  V [k, e] = x @ Wv.T          (lhsT = xT chunks,  rhs = WvT)
  ST[k, q] = K Q.T             (lhsT = KT chunks,  rhs = QT)   -> exp -> PT
  OT[e, q] = V.T P.T           (lhsT = V chunks,   rhs = PT)
Softmax runs without max-subtraction (scores are ~N(0, 0.2^2), so exp
cannot overflow and the result is mathematically identical).

Precision: Q/K are stored as fp8 e4m3 and the score matmul runs in
MatmulPerfMode.DoubleRow (2 contraction rows per pass, ~1.4x PE rate);
projections and P@V stay bf16 (fp32 PSUM accumulation everywhere).
Measured rel err vs the fp32 reference ~1.5e-2 (budget 2e-2).

Schedule: V projection runs first and its (bf16) AllGather is issued
before K's (fp8, half the bytes) so partner K/V arrive well before the
partner-half score/OT blocks need them. Score psum tiles hold two
k-tiles (2 PSUM banks) so one exp ACT covers 1024 columns, keeping the
scalar engine off the critical path. Row sums of P accumulate on the
Vector engine, are partition-reduced by one all-ones matmul per
q-chunk, and normalization + bv happen during OT evacuation (exact:
P@(V + 1*bv)/rowsum = P@V/rowsum + bv). Output is written as bf16.
A burst of throwaway matmuls warms the PE HAM clock gate during the
initial DMA wait.
"""

import sys

for _p in ("/opt/trn_rl_repo", "/root/.axon_site/_ro/trn_rl_repo"):
    if _p not in sys.path:
        sys.path.append(_p)

import numpy as np
import ml_dtypes

B, S, D = 4, 4096, 512
NCORES = 8
SQ = S // 2  # query rows per core
P = 128
NQ = 512  # q-chunk width (moving free dim)
DT = D // P  # 4 contraction chunks for d
ET = D // P  # 4 e tiles
KTI = S // P  # 32 k tiles
QC = SQ // NQ  # 4 q chunks per core
HKT = KTI // 2  # 16 local k-tiles per core
HS = S // 2  # 2048 local keys per core
SCALE = 1.0 / float(np.sqrt(np.float32(D)))

_CACHE = {}


def _split_excess_waits(nc, mybir, max_waits=1):
    """This walrus build rejects instructions carrying more than a couple of
    sync waits. Cap every instruction at `max_waits`, spilling the rest onto
    same-engine InstNoOps inserted immediately before it (equivalent
    semantics: the engine's stream stalls at the nop instead)."""
    for f in nc.m.functions:
        for bb in f.blocks:
            old = list(bb.instructions)
            new = []
            for inst in old:
                si = inst.sync_info
                waits = list(si.on_wait) if si is not None and si.on_wait else []
                if len(waits) > max_waits:
                    keep = waits[-max_waits:]
                    excess = waits[:-max_waits]
                    for i in range(0, len(excess), max_waits):
                        nop = mybir.InstNoOp(
                            name=f"waitnop-{nc.next_id()}", ins=[], outs=[]
                        )
                        nop.engine = inst.engine
                        nop.sync_info = mybir.SyncInfo(
                            on_wait=excess[i : i + max_waits], on_update=[]
                        )
                        new.append(nop)
                    inst.sync_info = mybir.SyncInfo(
                        on_wait=keep,
                        on_update=list(si.on_update) if si.on_update else [],
                    )
                new.append(inst)
            if len(new) != len(old):
                bb.instructions[:] = new


def _build_nc():
    import concourse.bass as bass
    import concourse.mybir as mybir
    import concourse.tile as tile
    from contextlib import ExitStack

    bf = mybir.dt.bfloat16
    f32 = mybir.dt.float32
    f8 = mybir.dt.float8e4
    AF = mybir.ActivationFunctionType
    DR = mybir.MatmulPerfMode.DoubleRow

    u32 = mybir.dt.uint32
    nc = bass.Bass()
    xT = nc.declare_dram_parameter("xT", [D, SQ], bf, isOutput=False)
    wqT = nc.declare_dram_parameter("wqT", [D, D], bf, isOutput=False)
    wkT = nc.declare_dram_parameter("wkT", [D, D], bf, isOutput=False)
    wvT = nc.declare_dram_parameter("wvT", [D, D], bf, isOutput=False)
    bqp = nc.declare_dram_parameter("bq", [P, ET], f32, isOutput=False)
    bkp = nc.declare_dram_parameter("bk", [P, ET], f32, isOutput=False)
    bvp = nc.declare_dram_parameter("bv", [P, ET], f32, isOutput=False)
    # Host-computed row bases into the AllGather outputs for the PARTNER
    # half (rank-dependent: (1-h)*512 + e*128 for KT, (1-h)*2048 + j*128
    # for V). Drives dynamic (register-offset) DMAs.
    poffp = nc.declare_dram_parameter("poff", [1, 6], u32, isOutput=False)
    ot = nc.declare_dram_parameter("ot", [D, SQ], bf, isOutput=True)

    with tile.TileContext(nc) as tc, ExitStack() as ctx:
        const_pool = ctx.enter_context(tc.tile_pool(name="const", bufs=1))
        persist = ctx.enter_context(tc.tile_pool(name="persist", bufs=1))
        outp = ctx.enter_context(tc.tile_pool(name="outp", bufs=2))

        ones = const_pool.tile([P, P], bf, tag="ones")
        nc.vector.memset(ones, 1.0)
        bq_sb = const_pool.tile([P, ET], f32, tag="bq")
        bk_sb = const_pool.tile([P, ET], f32, tag="bk")
        bv_sb = const_pool.tile([P, ET], f32, tag="bv")
        # Q/K in fp8 pair layout [p, e_sub, cols] for DoubleRow matmuls.
        qt_sb = persist.tile([P, ET, SQ], f8, tag="qt", name="qt")
        # K/V k-order per core: [my half, partner half]. Separate tiles per
        # half so partner DMA-writes create no false deps on local reads.
        kt_loc = persist.tile([P, ET, HS], f8, tag="ktl", name="ktl")
        kt_rem = persist.tile([P, ET, HS], f8, tag="ktr", name="ktr")
        v_loc = [persist.tile([P, D], bf, tag=f"v{k}", name=f"v{k}") for k in range(HKT)]
        v_rem = persist.tile([P, HKT, D], bf, tag="vr", name="vr")
        poff_sb = const_pool.tile([1, 6], mybir.dt.uint32, tag="poff")

        # ---- Phase 1+2: load inputs, project V + local-half K, AllGather
        # the V/K halves within each core pair, project Q ----
        with (
            tc.tile_pool(name="psA", bufs=4, space="PSUM") as psA,
            tc.tile_pool(name="xw", bufs=1) as xw_pool,
            tc.tile_pool(name="dram", bufs=1, space="DRAM") as dram,
        ):
            # x/w live only through the projections; closing this pool
            # frees their SBUF for the phase-3 PT tiles.
            wq_sb = xw_pool.tile([P, DT, D], bf, tag="wq", name="wq")
            wk_sb = xw_pool.tile([P, DT, D], bf, tag="wk", name="wk")
            wv_sb = xw_pool.tile([P, DT, D], bf, tag="wv", name="wv")
            x_sb = xw_pool.tile([P, DT, HS], bf, tag="x", name="x")
            vl_d = dram.tile([HKT * P, D], bf, tag="vl_d")
            vg_d = dram.tile([2 * HKT * P, D], bf, tag="vg_d")
            ktl_d = dram.tile([ET * P, HS], f8, tag="ktl_d")
            ktg_d = dram.tile([2 * ET * P, HS], f8, tag="ktg_d")

            # x column-chunked in first-consumer order on the SP HW queue;
            # weights (wk first: K runs first) on the ACT queue; biases on
            # the gpsimd queue. Each dma_start lands on one HW queue at
            # ~110 GB/s, so the critical early loads are split across
            # several issues to run queues in parallel.
            for d in range(DT):
                nc.sync.dma_start(
                    out=x_sb[:, d, :NQ], in_=xT[d * P : (d + 1) * P, :NQ]
                )
            for d in range(DT):
                nc.scalar.dma_start(
                    out=wk_sb[:, d, :], in_=wkT[d * P : (d + 1) * P, :]
                )
            nc.gpsimd.dma_start(out=bk_sb, in_=bkp[:, :])
            nc.gpsimd.dma_start(out=bv_sb, in_=bvp[:, :])
            nc.gpsimd.dma_start(out=bq_sb, in_=bqp[:, :])
            nc.gpsimd.dma_start(out=poff_sb, in_=poffp[:, :])
            for kc in range(1, QC):
                for dd in range(2):
                    nc.sync.dma_start(
                        out=x_sb[:, 2 * dd : 2 * dd + 2, kc * NQ : (kc + 1) * NQ],
                        in_=xT[
                            2 * dd * P : (2 * dd + 2) * P,
                            kc * NQ : (kc + 1) * NQ,
                        ].rearrange("(d p) c -> p d c", p=P),
                    )
            for dd in range(2):
                nc.scalar.dma_start(
                    out=wv_sb[:, 2 * dd : 2 * dd + 2, :],
                    in_=wvT[2 * dd * P : (2 * dd + 2) * P, :].rearrange(
                        "(d p) e -> p d e", p=P
                    ),
                )
            for dd in range(2):
                nc.scalar.dma_start(
                    out=wq_sb[:, 2 * dd : 2 * dd + 2, :],
                    in_=wqT[2 * dd * P : (2 * dd + 2) * P, :].rearrange(
                        "(d p) e -> p d e", p=P
                    ),
                )

            # Warm the PE HAM clock gate (~3.4us of activity flips it from
            # 1.2 to 2.4 GHz) with throwaway matmuls while the first input
            # DMAs are still in flight.
            warm_ps = psA.tile([P, P], f32, tag="warm", name="warm_ps", bufs=1)
            for _ in range(40):
                nc.tensor.matmul(warm_ps, lhsT=ones, rhs=ones, start=True, stop=True)

            pairs = [[2 * i, 2 * i + 1] for i in range(NCORES // 2)]

            # KT local half [e, 0:2048] -> fp8 (bias bk fused on evacuation).
            # K runs first: its AllGather result is needed first (partner
            # score blocks), and the two gathers serialize on one CC stream.
            for kc in range(QC):
                for e in range(ET):
                    ps = psA.tile([P, NQ], f32, tag="ps")
                    for d in range(DT):
                        nc.tensor.matmul(
                            ps,
                            lhsT=wk_sb[:, d, e * P : (e + 1) * P],
                            rhs=x_sb[:, d, kc * NQ : (kc + 1) * NQ],
                            start=(d == 0),
                            stop=(d == DT - 1),
                        )
                    nc.scalar.activation(
                        out=kt_loc[:, e, kc * NQ : (kc + 1) * NQ],
                        in_=ps,
                        func=AF.Identity,
                        bias=bk_sb[:, e : e + 1],
                        scale=1.0,
                    )
            for e in range(ET):
                nc.sync.dma_start(
                    out=ktl_d[e * P : (e + 1) * P, :], in_=kt_loc[:, e, :]
                )
            nc.gpsimd.collective_compute(
                "AllGather",
                mybir.AluOpType.bypass,
                replica_groups=pairs,
                ins=[ktl_d.opt()],
                outs=[ktg_d.opt()],
            )

            # V local half, tiles 0..15 (no bias; bv folded in at the end)
            for k in range(HKT):
                ps = psA.tile([P, D], f32, tag="ps")
                for d in range(DT):
                    nc.tensor.matmul(
                        ps,
                        lhsT=x_sb[:, d, k * P : (k + 1) * P],
                        rhs=wv_sb[:, d, :],
                        start=(d == 0),
                        stop=(d == DT - 1),
                    )
                nc.vector.tensor_copy(out=v_loc[k], in_=ps)
                nc.sync.dma_start(out=vl_d[k * P : (k + 1) * P, :], in_=v_loc[k])

            nc.gpsimd.collective_compute(
                "AllGather",
                mybir.AluOpType.bypass,
                replica_groups=pairs,
                ins=[vl_d.opt()],
                outs=[vg_d.opt()],
            )

            # QT[e, q] -> fp8 (bias bq fused on evacuation)
            for qc in range(QC):
                for e in range(ET):
                    ps = psA.tile([P, NQ], f32, tag="ps")
                    for d in range(DT):
                        nc.tensor.matmul(
                            ps,
                            lhsT=wq_sb[:, d, e * P : (e + 1) * P],
                            rhs=x_sb[:, d, qc * NQ : (qc + 1) * NQ],
                            start=(d == 0),
                            stop=(d == DT - 1),
                        )
                    nc.scalar.activation(
                        out=qt_sb[:, e, qc * NQ : (qc + 1) * NQ],
                        in_=ps,
                        func=AF.Identity,
                        bias=bq_sb[:, e : e + 1],
                        scale=1.0,
                    )

            # Partner-half loads from the gather outputs. Row bases are
            # rank-dependent, supplied by the host via `poff` and applied as
            # dynamic (register) offsets. K first: it is consumed first.
            # Split into pieces so several HW queues move them in parallel.
            SP = [mybir.EngineType.SP]
            for i in range(2):
                kt_base = nc.values_load(
                    poff_sb[0:1, i : i + 1], engines=SP,
                    min_val=0, max_val=2 * ET * P - 2 * P,
                    skip_runtime_bounds_check=True,
                )
                nc.sync.dma_start(
                    out=kt_rem[:, 2 * i : 2 * i + 2, :],
                    in_=ktg_d[bass.ds(kt_base, 2 * P), :].rearrange(
                        "(e p) c -> p e c", p=P
                    ),
                )
            for i in range(4):
                v_base = nc.values_load(
                    poff_sb[0:1, 2 + i : 3 + i], engines=SP,
                    min_val=0, max_val=2 * HKT * P - 4 * P,
                    skip_runtime_bounds_check=True,
                )
                nc.sync.dma_start(
                    out=v_rem[:, 4 * i : 4 * i + 4, :],
                    in_=vg_d[bass.ds(v_base, 4 * P), :].rearrange(
                        "(j p) c -> p j c", p=P
                    ),
                )

        # ---- Phase 3: attention ----
        # Static emission order staggers local-half score blocks ahead of
        # partner-half blocks so the PE has work while the AllGather +
        # partner DMAs are in flight. Score psum tiles hold 2 k-tiles so
        # one exp ACT covers 1024 columns.
        with (
            tc.tile_pool(name="pt", bufs=1) as pt_pool,
            tc.tile_pool(name="ps_st", bufs=2, space="PSUM") as ps_st,
            tc.tile_pool(name="ps_ot", bufs=4, space="PSUM") as ps_ot,
        ):
            ptl_tiles = {}
            ptp_tiles = {}
            rs_accs = {}

            def pt_slice(qc, k):
                if k < HKT:
                    return ptl_tiles[qc][:, k, :]
                return ptp_tiles[qc][:, k - HKT, :]

            def pt_slice2(qc, k):
                if k < HKT:
                    return ptl_tiles[qc][:, k : k + 2, :]
                return ptp_tiles[qc][:, k - HKT : k - HKT + 2, :]

            def st_alloc(qc, k0):
                if k0 == 0:
                    ptl_tiles[qc] = pt_pool.tile(
                        [P, HKT, NQ], bf, tag="ptl", name=f"ptl{qc}", bufs=4
                    )
                else:
                    ptp_tiles[qc] = pt_pool.tile(
                        [P, HKT, NQ], bf, tag="ptp", name=f"ptp{qc}", bufs=3
                    )

            def st_group(qc, k):
                # One 2-k-tile group: 4 DoubleRow matmuls -> one 1024-wide
                # exp -> two DVE rowsum adds.
                qsl = slice(qc * NQ, (qc + 1) * NQ)
                ps = ps_st.tile([P, 2, NQ], f32, tag="st", name="st_ps")
                for j in range(2):
                    kk = k + j
                    src = kt_loc if kk < HKT else kt_rem
                    ko = kk if kk < HKT else kk - HKT
                    for h in range(2):
                        nc.tensor.matmul(
                            ps[:, j, :],
                            lhsT=src[:, 2 * h : 2 * h + 2, ko * P : (ko + 1) * P],
                            rhs=qt_sb[:, 2 * h : 2 * h + 2, qsl],
                            start=(h == 0),
                            stop=(h == 1),
                            perf_mode=DR,
                        )
                nc.scalar.activation(
                    out=pt_slice2(qc, k), in_=ps, func=AF.Exp, scale=SCALE
                )
                if k == 0:
                    rs_accs[qc] = outp.tile(
                        [P, NQ], f32, tag="rs_acc", name=f"rs_acc{qc}", bufs=4
                    )
                    nc.vector.tensor_copy(out=rs_accs[qc], in_=pt_slice(qc, 0))
                    nc.vector.tensor_add(rs_accs[qc], rs_accs[qc], pt_slice(qc, 1))
                else:
                    nc.vector.tensor_add(rs_accs[qc], rs_accs[qc], pt_slice(qc, k))
                    nc.vector.tensor_add(
                        rs_accs[qc], rs_accs[qc], pt_slice(qc, k + 1)
                    )

            def st_block(qc, k0, k1):
                st_alloc(qc, k0)
                for k in range(k0, k1, 2):
                    st_group(qc, k)

            recips = {}

            def prep(qc):
                # Partition-reduce + replicate the DVE rowsum partials with
                # one all-ones matmul, then take the reciprocal on DVE.
                # Emitted well before fin_ot(qc) so the ~3.4us DVE divide is
                # off the critical path.
                rs_bf = outp.tile([P, NQ], bf, tag="rs_bf", bufs=1)
                nc.vector.tensor_copy(out=rs_bf, in_=rs_accs[qc])
                rs_ps = ps_ot.tile([P, NQ], f32, tag="ot", name="rs_ps")
                nc.tensor.matmul(rs_ps, lhsT=ones, rhs=rs_bf, start=True, stop=True)
                recips[qc] = outp.tile(
                    [P, NQ], f32, tag="recip", name=f"recip{qc}"
                )
                nc.vector.reciprocal(recips[qc], rs_ps)

            def evac(qc, e, ops, nh=1):
                # Normalize + add bv during evacuation. nh=2 drains in
                # halves so the DVE->ACT->DMA chain after the final matmul
                # is shorter.
                recip = recips[qc]
                HW = NQ // nh
                for i in range(nh):
                    csl = slice(i * HW, (i + 1) * HW)
                    tmp = outp.tile([P, HW], f32, tag=f"tmp{nh}", bufs=3)
                    nc.vector.tensor_mul(tmp, ops[:, csl], recip[:, csl])
                    tmpb = outp.tile([P, HW], bf, tag=f"tmpb{nh}", bufs=3)
                    nc.scalar.activation(
                        out=tmpb,
                        in_=tmp,
                        func=AF.Identity,
                        bias=bv_sb[:, e : e + 1],
                        scale=1.0,
                    )
                    nc.sync.dma_start(
                        out=ot[
                            e * P : (e + 1) * P,
                            qc * NQ + i * HW : qc * NQ + (i + 1) * HW,
                        ],
                        in_=tmpb,
                    )

            def ot_mms(qc, e, ops, kr):
                for k in kr:
                    if k < HKT:
                        vlhsT = v_loc[k][:, e * P : (e + 1) * P]
                    else:
                        vlhsT = v_rem[:, k - HKT, e * P : (e + 1) * P]
                    nc.tensor.matmul(
                        ops,
                        lhsT=vlhsT,
                        rhs=pt_slice(qc, k),
                        start=(k == 0),
                        stop=(k == KTI - 1),
                    )

            def fin_ot(qc, tail=False, st_qc=None):
                # prep(qc) is injected after e=0's matmul group: by then the
                # rowsum adds have drained (no PE stall on the ones-matmul)
                # and the ~3.4us DVE reciprocal still finishes before the
                # first evacuation mul is needed. With st_qc set, the next
                # q-chunk's partner-half score groups are interleaved at a
                # 4:16 matmul ratio so their exps never gate the PE.
                if st_qc is not None:
                    st_alloc(st_qc, HKT)
                for e in range(ET):
                    if st_qc is not None:
                        st_group(st_qc, HKT + 4 * e)
                        st_group(st_qc, HKT + 4 * e + 2)
                    ops = ps_ot.tile([P, NQ], f32, tag="ot")
                    ot_mms(qc, e, ops, range(KTI))
                    if e == 0:
                        prep(qc)
                    evac(qc, e, ops, nh=2 if (tail and e >= ET - 2) else 1)

            st_block(0, 0, HKT)
            st_block(1, 0, HKT)
            st_block(2, 0, HKT)
            st_block(3, 0, HKT)
            st_block(0, HKT, KTI)
            st_block(1, HKT, KTI)
            fin_ot(0)
            st_block(2, HKT, KTI)
            fin_ot(1)
            st_block(3, HKT, KTI)
            fin_ot(2)
            fin_ot(3, tail=True)

    _split_excess_waits(nc, mybir)
    return nc


def _get_nc():
    if "nc" not in _CACHE:
        _CACHE["nc"] = _build_nc()
    return _CACHE["nc"]


def _make_in_maps(x, Wq, bq, Wk, bk, Wv, bv):
    bf16 = ml_dtypes.bfloat16
    wqT = np.ascontiguousarray(Wq.T).astype(bf16)
    wkT = np.ascontiguousarray(Wk.T).astype(bf16)
    wvT = np.ascontiguousarray(Wv.T).astype(bf16)
    bqp = np.ascontiguousarray(bq.reshape(ET, P).T).astype(np.float32)
    bkp = np.ascontiguousarray(bk.reshape(ET, P).T).astype(np.float32)
    bvp = np.ascontiguousarray(bv.reshape(ET, P).T).astype(np.float32)
    in_maps = []
    for c in range(NCORES):
        b, h = divmod(c, 2)
        # Local half of x[b].T: both this core's query columns and its K/V
        # half (they are the same row range by construction).
        xTl = np.ascontiguousarray(x[b, h * SQ : (h + 1) * SQ, :].T).astype(bf16)
        # Partner-half row bases into the rank-ordered AllGather outputs:
        # 2 pieces (e-pairs) for KT, 4 pieces (4 k-tiles each) for V.
        kb = (1 - h) * ET * P
        vb = (1 - h) * HKT * P
        poff = np.array(
            [[kb, kb + 2 * P, vb, vb + 4 * P, vb + 8 * P, vb + 12 * P]],
            dtype=np.uint32,
        )
        in_maps.append(
            {
                "xT": xTl,
                "poff": poff,
                "wqT": wqT,
                "wkT": wkT,
                "wvT": wvT,
                "bq": bqp,
                "bk": bkp,
                "bv": bvp,
            }
        )
    return in_maps


def _run(in_maps, **kwargs):
    from concourse.bass_utils import run_bass_kernel_spmd

    nc = _get_nc()
    return run_bass_kernel_spmd(nc, in_maps, core_ids=list(range(NCORES)), **kwargs)


def kernel(x, Wq, bq, Wk, bk, Wv, bv):
    x = np.asarray(x, dtype=np.float32)
    Wq = np.asarray(Wq, dtype=np.float32)
    Wk = np.asarray(Wk, dtype=np.float32)
    Wv = np.asarray(Wv, dtype=np.float32)
    bq = np.asarray(bq, dtype=np.float32)
    bk = np.asarray(bk, dtype=np.float32)
    bv = np.asarray(bv, dtype=np.float32)

    res = _run(_make_in_maps(x, Wq, bq, Wk, bk, Wv, bv))
    out = np.empty((B, S, D), dtype=np.float32)
    for c in range(NCORES):
        b, h = divmod(c, 2)
        out[b, h * SQ : (h + 1) * SQ, :] = (
            np.asarray(res.results[c]["ot"]).astype(np.float32).T
        )
    return out


# revision 35
# speedup vs baseline: 1.0197x; 1.0197x over previous
"""BertSelfAttention (B=4, S=4096, D=512) on 8 TRN2 NeuronCores.

Sharding: core c handles batch b = c//2 and query-row half h = c%2
(2048 q rows). Each core projects K/V for its own 2048 keys and the
halves are exchanged within each core pair via AllGather.

Layout trick: everything is computed transposed so no on-device
transposes are needed:
  QT[e, q] = Wq @ x.T          (lhsT = WqT chunks, rhs = xT chunks)
  KT[e, k] = Wk @ x.T
  V [k, e] = x @ Wv.T          (lhsT = xT chunks,  rhs = WvT)
  ST[k, q] = K Q.T             (lhsT = KT chunks,  rhs = QT)   -> exp -> PT
  OT[e, q] = V.T P.T           (lhsT = V chunks,   rhs = PT)
Softmax runs without max-subtraction (scores are ~N(0, 0.2^2), so exp
cannot overflow and the result is mathematically identical).

Precision: Q/K are stored as fp8 e4m3 and the score matmul runs in
MatmulPerfMode.DoubleRow (2 contraction rows per pass, ~1.4x PE rate);
projections and P@V stay bf16 (fp32 PSUM accumulation everywhere).
Measured rel err vs the fp32 reference ~1.5e-2 (budget 2e-2).

Schedule: V projection runs first and its (bf16) AllGather is issued
before K's (fp8, half the bytes) so partner K/V arrive well before the
partner-half score/OT blocks need them. Score psum tiles hold two
k-tiles (2 PSUM banks) so one exp ACT covers 1024 columns, keeping the
scalar engine off the critical path. Row sums of P accumulate on the
Vector engine, are partition-reduced by one all-ones matmul per
q-chunk, and normalization + bv happen during OT evacuation (exact:
P@(V + 1*bv)/rowsum = P@V/rowsum + bv). Output is written as bf16.
A burst of throwaway matmuls warms the PE HAM clock gate during the
initial DMA wait.
"""

import sys

for _p in ("/opt/trn_rl_repo", "/root/.axon_site/_ro/trn_rl_repo"):
    if _p not in sys.path:
        sys.path.append(_p)

import numpy as np
import ml_dtypes

B, S, D = 4, 4096, 512
NCORES = 8
SQ = S // 2  # query rows per core
P = 128
NQ = 512  # q-chunk width (moving free dim)
DT = D // P  # 4 contraction chunks for d
ET = D // P  # 4 e tiles
KTI = S // P  # 32 k tiles
QC = SQ // NQ  # 4 q chunks per core
HKT = KTI // 2  # 16 local k-tiles per core
HS = S // 2  # 2048 local keys per core
SCALE = 1.0 / float(np.sqrt(np.float32(D)))

_CACHE = {}


def _split_excess_waits(nc, mybir, max_waits=1):
    """This walrus build rejects instructions carrying more than a couple of
    sync waits. Cap every instruction at `max_waits`, spilling the rest onto
    same-engine InstNoOps inserted immediately before it (equivalent
    semantics: the engine's stream stalls at the nop instead)."""
    for f in nc.m.functions:
        for bb in f.blocks:
            old = list(bb.instructions)
            new = []
            for inst in old:
                si = inst.sync_info
                waits = list(si.on_wait) if si is not None and si.on_wait else []
                if len(waits) > max_waits:
                    keep = waits[-max_waits:]
                    excess = waits[:-max_waits]
                    for i in range(0, len(excess), max_waits):
                        nop = mybir.InstNoOp(
                            name=f"waitnop-{nc.next_id()}", ins=[], outs=[]
                        )
                        nop.engine = inst.engine
                        nop.sync_info = mybir.SyncInfo(
                            on_wait=excess[i : i + max_waits], on_update=[]
                        )
                        new.append(nop)
                    inst.sync_info = mybir.SyncInfo(
                        on_wait=keep,
                        on_update=list(si.on_update) if si.on_update else [],
                    )
                new.append(inst)
            if len(new) != len(old):
                bb.instructions[:] = new


def _build_nc():
    import concourse.bass as bass
    import concourse.mybir as mybir
    import concourse.tile as tile
    from contextlib import ExitStack

    bf = mybir.dt.bfloat16
    f32 = mybir.dt.float32
    f8 = mybir.dt.float8e4
    AF = mybir.ActivationFunctionType
    DR = mybir.MatmulPerfMode.DoubleRow

    u32 = mybir.dt.uint32
    nc = bass.Bass()
    xT = nc.declare_dram_parameter("xT", [D, SQ], bf, isOutput=False)
    wqT = nc.declare_dram_parameter("wqT", [D, D], bf, isOutput=False)
    wkT = nc.declare_dram_parameter("wkT", [D, D], bf, isOutput=False)
    wvT = nc.declare_dram_parameter("wvT", [D, D], bf, isOutput=False)
    bqp = nc.declare_dram_parameter("bq", [P, ET], f32, isOutput=False)
    bkp = nc.declare_dram_parameter("bk", [P, ET], f32, isOutput=False)
    bvp = nc.declare_dram_parameter("bv", [P, ET], f32, isOutput=False)
    # Host-computed row bases into the AllGather outputs for the PARTNER
    # half (rank-dependent: (1-h)*512 + e*128 for KT, (1-h)*2048 + j*128
    # for V). Drives dynamic (register-offset) DMAs.
    poffp = nc.declare_dram_parameter("poff", [1, 6], u32, isOutput=False)
    ot = nc.declare_dram_parameter("ot", [D, SQ], bf, isOutput=True)

    with tile.TileContext(nc) as tc, ExitStack() as ctx:
        const_pool = ctx.enter_context(tc.tile_pool(name="const", bufs=1))
        persist = ctx.enter_context(tc.tile_pool(name="persist", bufs=1))
        outp = ctx.enter_context(tc.tile_pool(name="outp", bufs=2))

        ones = const_pool.tile([P, P], bf, tag="ones")
        nc.vector.memset(ones, 1.0)
        bq_sb = const_pool.tile([P, ET], f32, tag="bq")
        bk_sb = const_pool.tile([P, ET], f32, tag="bk")
        bv_sb = const_pool.tile([P, ET], f32, tag="bv")
        # Q/K in fp8 pair layout [p, e_sub, cols] for DoubleRow matmuls.
        qt_sb = persist.tile([P, ET, SQ], f8, tag="qt", name="qt")
        # K/V k-order per core: [my half, partner half]. Separate tiles per
        # half so partner DMA-writes create no false deps on local reads.
        kt_loc = persist.tile([P, ET, HS], f8, tag="ktl", name="ktl")
        kt_rem = persist.tile([P, ET, HS], f8, tag="ktr", name="ktr")
        v_loc = [persist.tile([P, D], bf, tag=f"v{k}", name=f"v{k}") for k in range(HKT)]
        v_rem = persist.tile([P, HKT, D], bf, tag="vr", name="vr")
        poff_sb = const_pool.tile([1, 6], mybir.dt.uint32, tag="poff")

        # ---- Phase 1+2: load inputs, project V + local-half K, AllGather
        # the V/K halves within each core pair, project Q ----
        with (
            tc.tile_pool(name="psA", bufs=4, space="PSUM") as psA,
            tc.tile_pool(name="xw", bufs=1) as xw_pool,
            tc.tile_pool(name="dram", bufs=1, space="DRAM") as dram,
        ):
            # x/w live only through the projections; closing this pool
            # frees their SBUF for the phase-3 PT tiles.
            wq_sb = xw_pool.tile([P, DT, D], bf, tag="wq", name="wq")
            wk_sb = xw_pool.tile([P, DT, D], bf, tag="wk", name="wk")
            wv_sb = xw_pool.tile([P, DT, D], bf, tag="wv", name="wv")
            x_sb = xw_pool.tile([P, DT, HS], bf, tag="x", name="x")
            vl_d = dram.tile([HKT * P, D], bf, tag="vl_d")
            vg_d = dram.tile([2 * HKT * P, D], bf, tag="vg_d")
            ktl_d = dram.tile([ET * P, HS], f8, tag="ktl_d")
            ktg_d = dram.tile([2 * ET * P, HS], f8, tag="ktg_d")

            # x column-chunked in first-consumer order on the SP HW queue;
            # weights (wk first: K runs first) on the ACT queue; biases on
            # the gpsimd queue. Each dma_start lands on one HW queue at
            # ~110 GB/s, so the critical early loads are split across
            # several issues to run queues in parallel.
            for d in range(DT):
                nc.sync.dma_start(
                    out=x_sb[:, d, :NQ], in_=xT[d * P : (d + 1) * P, :NQ]
                )
            for d in range(DT):
                nc.scalar.dma_start(
                    out=wk_sb[:, d, :], in_=wkT[d * P : (d + 1) * P, :]
                )
            nc.gpsimd.dma_start(out=bk_sb, in_=bkp[:, :])
            nc.gpsimd.dma_start(out=bv_sb, in_=bvp[:, :])
            nc.gpsimd.dma_start(out=bq_sb, in_=bqp[:, :])
            nc.gpsimd.dma_start(out=poff_sb, in_=poffp[:, :])
            for kc in range(1, QC):
                for dd in range(2):
                    nc.sync.dma_start(
                        out=x_sb[:, 2 * dd : 2 * dd + 2, kc * NQ : (kc + 1) * NQ],
                        in_=xT[
                            2 * dd * P : (2 * dd + 2) * P,
                            kc * NQ : (kc + 1) * NQ,
                        ].rearrange("(d p) c -> p d c", p=P),
                    )
            for dd in range(2):
                nc.scalar.dma_start(
                    out=wv_sb[:, 2 * dd : 2 * dd + 2, :],
                    in_=wvT[2 * dd * P : (2 * dd + 2) * P, :].rearrange(
                        "(d p) e -> p d e", p=P
                    ),
                )
            for dd in range(2):
                nc.scalar.dma_start(
                    out=wq_sb[:, 2 * dd : 2 * dd + 2, :],
                    in_=wqT[2 * dd * P : (2 * dd + 2) * P, :].rearrange(
                        "(d p) e -> p d e", p=P
                    ),
                )

            # Warm the PE HAM clock gate (~3.4us of activity flips it from
            # 1.2 to 2.4 GHz) with throwaway matmuls while the first input
            # DMAs are still in flight.
            warm_ps = psA.tile([P, P], f32, tag="warm", name="warm_ps", bufs=1)
            for _ in range(40):
                nc.tensor.matmul(warm_ps, lhsT=ones, rhs=ones, start=True, stop=True)

            pairs = [[2 * i, 2 * i + 1] for i in range(NCORES // 2)]

            # KT local half [e, 0:2048] -> fp8 (bias bk fused on evacuation).
            # K runs first: its AllGather result is needed first (partner
            # score blocks), and the two gathers serialize on one CC stream.
            for kc in range(QC):
                for e in range(ET):
                    ps = psA.tile([P, NQ], f32, tag="ps")
                    for d in range(DT):
                        nc.tensor.matmul(
                            ps,
                            lhsT=wk_sb[:, d, e * P : (e + 1) * P],
                            rhs=x_sb[:, d, kc * NQ : (kc + 1) * NQ],
                            start=(d == 0),
                            stop=(d == DT - 1),
                        )
                    nc.scalar.activation(
                        out=kt_loc[:, e, kc * NQ : (kc + 1) * NQ],
                        in_=ps,
                        func=AF.Identity,
                        bias=bk_sb[:, e : e + 1],
                        scale=1.0,
                    )
            for e in range(ET):
                nc.sync.dma_start(
                    out=ktl_d[e * P : (e + 1) * P, :], in_=kt_loc[:, e, :]
                )
            nc.gpsimd.collective_compute(
                "AllGather",
                mybir.AluOpType.bypass,
                replica_groups=pairs,
                ins=[ktl_d.opt()],
                outs=[ktg_d.opt()],
            )

            # V local half, tiles 0..15 (no bias; bv folded in at the end)
            for k in range(HKT):
                ps = psA.tile([P, D], f32, tag="ps")
                for d in range(DT):
                    nc.tensor.matmul(
                        ps,
                        lhsT=x_sb[:, d, k * P : (k + 1) * P],
                        rhs=wv_sb[:, d, :],
                        start=(d == 0),
                        stop=(d == DT - 1),
                    )
                nc.vector.tensor_copy(out=v_loc[k], in_=ps)
                nc.sync.dma_start(out=vl_d[k * P : (k + 1) * P, :], in_=v_loc[k])

            nc.gpsimd.collective_compute(
                "AllGather",
                mybir.AluOpType.bypass,
                replica_groups=pairs,
                ins=[vl_d.opt()],
                outs=[vg_d.opt()],
            )

            # QT[e, q] -> fp8 (bias bq fused on evacuation)
            for qc in range(QC):
                for e in range(ET):
                    ps = psA.tile([P, NQ], f32, tag="ps")
                    for d in range(DT):
                        nc.tensor.matmul(
                            ps,
                            lhsT=wq_sb[:, d, e * P : (e + 1) * P],
                            rhs=x_sb[:, d, qc * NQ : (qc + 1) * NQ],
                            start=(d == 0),
                            stop=(d == DT - 1),
                        )
                    nc.scalar.activation(
                        out=qt_sb[:, e, qc * NQ : (qc + 1) * NQ],
                        in_=ps,
                        func=AF.Identity,
                        bias=bq_sb[:, e : e + 1],
                        scale=1.0,
                    )

            # Partner-half loads from the gather outputs. Row bases are
            # rank-dependent, supplied by the host via `poff` and applied as
            # dynamic (register) offsets. K first: it is consumed first.
            # Split into pieces so several HW queues move them in parallel.
            SP = [mybir.EngineType.SP]
            for i in range(2):
                kt_base = nc.values_load(
                    poff_sb[0:1, i : i + 1], engines=SP,
                    min_val=0, max_val=2 * ET * P - 2 * P,
                    skip_runtime_bounds_check=True,
                )
                nc.sync.dma_start(
                    out=kt_rem[:, 2 * i : 2 * i + 2, :],
                    in_=ktg_d[bass.ds(kt_base, 2 * P), :].rearrange(
                        "(e p) c -> p e c", p=P
                    ),
                )
            for i in range(4):
                v_base = nc.values_load(
                    poff_sb[0:1, 2 + i : 3 + i], engines=SP,
                    min_val=0, max_val=2 * HKT * P - 4 * P,
                    skip_runtime_bounds_check=True,
                )
                nc.sync.dma_start(
                    out=v_rem[:, 4 * i : 4 * i + 4, :],
                    in_=vg_d[bass.ds(v_base, 4 * P), :].rearrange(
                        "(j p) c -> p j c", p=P
                    ),
                )

        # ---- Phase 3: attention ----
        # Static emission order staggers local-half score blocks ahead of
        # partner-half blocks so the PE has work while the AllGather +
        # partner DMAs are in flight. Score psum tiles hold 2 k-tiles so
        # one exp ACT covers 1024 columns.
        with (
            tc.tile_pool(name="pt", bufs=1) as pt_pool,
            tc.tile_pool(name="ps_st", bufs=2, space="PSUM") as ps_st,
            tc.tile_pool(name="ps_ot", bufs=4, space="PSUM") as ps_ot,
        ):
            ptl_tiles = {}
            ptp_tiles = {}
            rs_accs = {}

            def pt_slice(qc, k):
                if k < HKT:
                    return ptl_tiles[qc][:, k, :]
                return ptp_tiles[qc][:, k - HKT, :]

            def pt_slice2(qc, k):
                if k < HKT:
                    return ptl_tiles[qc][:, k : k + 2, :]
                return ptp_tiles[qc][:, k - HKT : k - HKT + 2, :]

            def st_alloc(qc, k0):
                if k0 == 0:
                    ptl_tiles[qc] = pt_pool.tile(
                        [P, HKT, NQ], bf, tag="ptl", name=f"ptl{qc}", bufs=4
                    )
                else:
                    ptp_tiles[qc] = pt_pool.tile(
                        [P, HKT, NQ], bf, tag="ptp", name=f"ptp{qc}", bufs=3
                    )

            def st_group(qc, k):
                # One 2-k-tile group: 4 DoubleRow matmuls -> one 1024-wide
                # exp -> two DVE rowsum adds.
                qsl = slice(qc * NQ, (qc + 1) * NQ)
                ps = ps_st.tile([P, 2, NQ], f32, tag="st", name="st_ps")
                for j in range(2):
                    kk = k + j
                    src = kt_loc if kk < HKT else kt_rem
                    ko = kk if kk < HKT else kk - HKT
                    for h in range(2):
                        nc.tensor.matmul(
                            ps[:, j, :],
                            lhsT=src[:, 2 * h : 2 * h + 2, ko * P : (ko + 1) * P],
                            rhs=qt_sb[:, 2 * h : 2 * h + 2, qsl],
                            start=(h == 0),
                            stop=(h == 1),
                            perf_mode=DR,
                        )
                nc.scalar.activation(
                    out=pt_slice2(qc, k), in_=ps, func=AF.Exp, scale=SCALE
                )
                if k == 0:
                    rs_accs[qc] = outp.tile(
                        [P, NQ], f32, tag="rs_acc", name=f"rs_acc{qc}", bufs=4
                    )
                    nc.vector.tensor_copy(out=rs_accs[qc], in_=pt_slice(qc, 0))
                    nc.vector.tensor_add(rs_accs[qc], rs_accs[qc], pt_slice(qc, 1))
                else:
                    nc.vector.tensor_add(rs_accs[qc], rs_accs[qc], pt_slice(qc, k))
                    nc.vector.tensor_add(
                        rs_accs[qc], rs_accs[qc], pt_slice(qc, k + 1)
                    )

            def st_block(qc, k0, k1):
                st_alloc(qc, k0)
                for k in range(k0, k1, 2):
                    st_group(qc, k)

            recips = {}

            def prep(qc):
                # Partition-reduce + replicate the DVE rowsum partials with
                # one all-ones matmul, then take the reciprocal on DVE.
                # Emitted well before fin_ot(qc) so the ~3.4us DVE divide is
                # off the critical path.
                rs_bf = outp.tile([P, NQ], bf, tag="rs_bf", bufs=1)
                nc.vector.tensor_copy(out=rs_bf, in_=rs_accs[qc])
                rs_big = ps_st.tile([P, 2, NQ], f32, tag="st", name="rs_big")
                rs_ps = rs_big[:, 0, :]
                nc.tensor.matmul(rs_ps, lhsT=ones, rhs=rs_bf, start=True, stop=True)
                recips[qc] = outp.tile(
                    [P, NQ], f32, tag="recip", name=f"recip{qc}"
                )
                nc.vector.reciprocal(recips[qc], rs_ps)

            def evac(qc, e, ops, nh=1):
                # Normalize + add bv during evacuation. nh=2 drains in
                # halves so the DVE->ACT->DMA chain after the final matmul
                # is shorter.
                recip = recips[qc]
                HW = NQ // nh
                for i in range(nh):
                    csl = slice(i * HW, (i + 1) * HW)
                    tmp = outp.tile([P, HW], f32, tag=f"tmp{nh}", bufs=3)
                    nc.vector.tensor_mul(tmp, ops[:, csl], recip[:, csl])
                    tmpb = outp.tile([P, HW], bf, tag=f"tmpb{nh}", bufs=3)
                    nc.scalar.activation(
                        out=tmpb,
                        in_=tmp,
                        func=AF.Identity,
                        bias=bv_sb[:, e : e + 1],
                        scale=1.0,
                    )
                    nc.sync.dma_start(
                        out=ot[
                            e * P : (e + 1) * P,
                            qc * NQ + i * HW : qc * NQ + (i + 1) * HW,
                        ],
                        in_=tmpb,
                    )

            def ot_mms(qc, e, ops, kr):
                for k in kr:
                    if k < HKT:
                        vlhsT = v_loc[k][:, e * P : (e + 1) * P]
                    else:
                        vlhsT = v_rem[:, k - HKT, e * P : (e + 1) * P]
                    nc.tensor.matmul(
                        ops,
                        lhsT=vlhsT,
                        rhs=pt_slice(qc, k),
                        start=(k == 0),
                        stop=(k == KTI - 1),
                    )

            def fin_ot(qc, tail=False, st_qc=None):
                # prep(qc) is injected after e=0's matmul group: by then the
                # rowsum adds have drained (no PE stall on the ones-matmul)
                # and the ~3.4us DVE reciprocal still finishes before the
                # first evacuation mul is needed. With st_qc set, the next
                # q-chunk's partner-half score groups are interleaved at a
                # 4:16 matmul ratio so their exps never gate the PE.
                if st_qc is not None:
                    st_alloc(st_qc, HKT)
                for e in range(ET):
                    if st_qc is not None:
                        st_group(st_qc, HKT + 4 * e)
                        st_group(st_qc, HKT + 4 * e + 2)
                    ops = ps_ot.tile([P, NQ], f32, tag="ot")
                    ot_mms(qc, e, ops, range(KTI))
                    if e == 0:
                        prep(qc)
                    evac(qc, e, ops, nh=2 if (tail and e >= ET - 2) else 1)

            st_block(0, 0, HKT)
            st_block(1, 0, HKT)
            st_block(2, 0, HKT)
            st_block(3, 0, HKT)
            st_block(0, HKT, KTI)
            st_block(1, HKT, KTI)
            fin_ot(0)
            st_block(2, HKT, KTI)
            fin_ot(1)
            st_block(3, HKT, KTI)
            fin_ot(2)
            fin_ot(3, tail=True)

    _split_excess_waits(nc, mybir)
    return nc


def _get_nc():
    if "nc" not in _CACHE:
        _CACHE["nc"] = _build_nc()
    return _CACHE["nc"]


def _make_in_maps(x, Wq, bq, Wk, bk, Wv, bv):
    bf16 = ml_dtypes.bfloat16
    wqT = np.ascontiguousarray(Wq.T).astype(bf16)
    wkT = np.ascontiguousarray(Wk.T).astype(bf16)
    wvT = np.ascontiguousarray(Wv.T).astype(bf16)
    bqp = np.ascontiguousarray(bq.reshape(ET, P).T).astype(np.float32)
    bkp = np.ascontiguousarray(bk.reshape(ET, P).T).astype(np.float32)
    bvp = np.ascontiguousarray(bv.reshape(ET, P).T).astype(np.float32)
    in_maps = []
    for c in range(NCORES):
        b, h = divmod(c, 2)
        # Local half of x[b].T: both this core's query columns and its K/V
        # half (they are the same row range by construction).
        xTl = np.ascontiguousarray(x[b, h * SQ : (h + 1) * SQ, :].T).astype(bf16)
        # Partner-half row bases into the rank-ordered AllGather outputs:
        # 2 pieces (e-pairs) for KT, 4 pieces (4 k-tiles each) for V.
        kb = (1 - h) * ET * P
        vb = (1 - h) * HKT * P
        poff = np.array(
            [[kb, kb + 2 * P, vb, vb + 4 * P, vb + 8 * P, vb + 12 * P]],
            dtype=np.uint32,
        )
        in_maps.append(
            {
                "xT": xTl,
                "poff": poff,
                "wqT": wqT,
                "wkT": wkT,
                "wvT": wvT,
                "bq": bqp,
                "bk": bkp,
                "bv": bvp,
            }
        )
    return in_maps


def _run(in_maps, **kwargs):
    from concourse.bass_utils import run_bass_kernel_spmd

    nc = _get_nc()
    return run_bass_kernel_spmd(nc, in_maps, core_ids=list(range(NCORES)), **kwargs)


def kernel(x, Wq, bq, Wk, bk, Wv, bv):
    x = np.asarray(x, dtype=np.float32)
    Wq = np.asarray(Wq, dtype=np.float32)
    Wk = np.asarray(Wk, dtype=np.float32)
    Wv = np.asarray(Wv, dtype=np.float32)
    bq = np.asarray(bq, dtype=np.float32)
    bk = np.asarray(bk, dtype=np.float32)
    bv = np.asarray(bv, dtype=np.float32)

    res = _run(_make_in_maps(x, Wq, bq, Wk, bk, Wv, bv))
    out = np.empty((B, S, D), dtype=np.float32)
    for c in range(NCORES):
        b, h = divmod(c, 2)
        out[b, h * SQ : (h + 1) * SQ, :] = (
            np.asarray(res.results[c]["ot"]).astype(np.float32).T
        )
    return out
